# revision 9
# baseline (speedup 1.0000x reference)
"""CfC head (mLSTM-style scan) Trainium2 kernel.

Math (per timestep t, per (b,h)):
    pre_g = xt*Wg_w + Wg_b            (xt = (x_codes-65)/100)
    i_t = exp(pre_i - n), f_t = exp(pre_f - n), o_t = exp(pre_o - n)
    g_t = sigmoid(pre_g); lam = sigmoid(pre_l)
    c   = f_t*c + i_t*g_t
    h   = (h + DT*o_t*sigmoid(c)) / (1 + DT*lam)
    n  += 0.01*(i_t + f_t + o_t - 3)
    y_t = h @ proj_w.T + proj_b

Device mapping: H=1024 sharded over 8 cores (128 h-values per core, one SBUF
partition each); free dim packs (batch-major, time-minor) blocks of TB steps.
The n-recurrence is handled per block by tracking the within-block drift
delta = n - n_blockstart, linearized as the affine scan
    delta_t = (1 - 0.01*P_t) * delta_{t-1} + (0.01*P_t - 0.03),
    P_t = (Ei+Ef+Eo)_t * exp(-n_blockstart),
which runs as one tensor_tensor_scan over the whole block (validated: rel err
1.4e-4 at TB=32 vs exact). c and h are exact affine scans given en = exp(-n):
    c_t = (Ef_t*en) * c_{t-1} + (Ei_t*G_t*en)
    h_t = L1_t * h_{t-1} + L1_t*DT*Eo_t*en*sigmoid(c_t),  L1 = 1/(1+DT*lam)
L1 uses the Neumann form 1 - q + q^2 = (q-0.5)^2 + 0.75 (q = DT*lam <= 0.01).
Sigmoids use tanh so every activation (exp/tanh/square) lives in the single
"exp_and_others" ACT table set (no table reloads).

y partials (over each core's 128 h) are accumulated on PE into PSUM and
summed across cores on the host.
"""

import os
from contextlib import ExitStack

import numpy as np

import concourse.bacc as bacc
import concourse.mybir as mybir
import concourse.tile as tile
from concourse.bass_utils import run_bass_kernel_spmd

AF = mybir.ActivationFunctionType
OP = mybir.AluOpType
F32 = mybir.dt.float32

B, S, H = 64, 2048, 1024
NCORES = 8
HC = H // NCORES  # 128 h-values per core = partition dim
DT = 0.01

TB = int(os.environ.get("KERNEL_TB", "32"))  # timesteps per block

_cached = {}
_last_results = None


def build_program(s=S, tb=TB):
    nb = s // tb
    nfd = B * tb           # free dim of block tiles, (b-major, t-minor)
    nslab = nfd // 128     # 128-wide matmul slabs per block

    nc = bacc.Bacc(
        "TRN2", target_bir_lowering=False, debug=False, num_devices=NCORES
    )
    x_d = nc.dram_tensor("x", [B, s], F32, kind="ExternalInput").ap()
    wv_d = nc.dram_tensor("wv", [HC, 10], F32, kind="ExternalInput").ap()
    pj_d = nc.dram_tensor("projT", [HC, 2], F32, kind="ExternalInput").ap()
    n0_d = nc.dram_tensor("n0", [HC, 1], F32, kind="ExternalInput").ap()
    y_d = nc.dram_tensor("yout", [nb, 128, tb], F32, kind="ExternalOutput").ap()

    def r3(ap):  # [128, nfd] -> [128, B, tb]
        return ap.rearrange("p (b t) -> p b t", t=tb)

    with tile.TileContext(nc) as tc, ExitStack() as ctx:
        wp = ctx.enter_context(tc.tile_pool(name="w", bufs=1))
        pha = ctx.enter_context(tc.tile_pool(name="pha", bufs=2))
        chn = ctx.enter_context(tc.tile_pool(name="chn", bufs=1))
        sm = ctx.enter_context(tc.tile_pool(name="sm", bufs=2))
        pp = ctx.enter_context(tc.tile_pool(name="pp", bufs=2, space="PSUM"))

        wv = wp.tile([HC, 10], F32)
        nc.sync.dma_start(wv[:], wv_d)
        pj = wp.tile([HC, 2], F32)
        nc.sync.dma_start(pj[:], pj_d)
        n0t = wp.tile([HC, 1], F32)
        nc.sync.dma_start(n0t[:], n0_d)

        # carries: n at block start (per h,b), exp(-n), c, h
        Nc = wp.tile([HC, B], F32)
        nc.vector.memset(Nc[:], 0.0)
        nc.vector.tensor_scalar(Nc[:], Nc[:], n0t[:, 0:1], None, OP.add)
        ENc = wp.tile([HC, B], F32)
        nc.scalar.activation(ENc[:], Nc[:], AF.Exp, scale=-1.0)
        cz = wp.tile([HC, B], F32)
        nc.vector.memset(cz[:], 0.0)
        hz = wp.tile([HC, B], F32)
        nc.vector.memset(hz[:], 0.0)
        Cc_v, Hc_v = cz[:], hz[:]
        bm05 = wp.tile([HC, 1], F32)
        nc.vector.memset(bm05[:], -0.5)

        for k in range(nb):
            t0 = k * tb
            X = pha.tile([128, nfd], F32, tag="X")
            nc.sync.dma_start(
                r3(X[:]), x_d[:, t0 : t0 + tb].partition_broadcast(128)
            )
            # gate pre-activations, fused through ACT scale/bias
            Ei = pha.tile([128, nfd], F32, tag="Ei")
            nc.scalar.activation(
                Ei[:], X[:], AF.Exp, bias=wv[:, 1:2], scale=wv[:, 0:1]
            )
            Ef = pha.tile([128, nfd], F32, tag="Ef")
            nc.scalar.activation(
                Ef[:], X[:], AF.Exp, bias=wv[:, 3:4], scale=wv[:, 2:3]
            )
            Eo = pha.tile([128, nfd], F32, tag="Eo")
            nc.scalar.activation(
                Eo[:], X[:], AF.Exp, bias=wv[:, 5:6], scale=wv[:, 4:5]
            )
            Tg = pha.tile([128, nfd], F32, tag="Tg")
            nc.scalar.activation(
                Tg[:], X[:], AF.Tanh, bias=wv[:, 7:8], scale=wv[:, 6:7]
            )
            Tl = pha.tile([128, nfd], F32, tag="Tl")
            nc.scalar.activation(
                Tl[:], X[:], AF.Tanh, bias=wv[:, 9:10], scale=wv[:, 8:9]
            )

            # G = 0.5*Tg+0.5 ; EiG = Ei*G  (both land in Tg; on GpSimd, which
            # is otherwise idle -- DVE is the kernel bottleneck)
            nc.gpsimd.tensor_scalar(Tg[:], Tg[:], 0.5, 0.5, OP.mult, OP.add)
            nc.gpsimd.tensor_mul(Tg[:], Ei[:], Tg[:])
            # Esum = Ei+Ef+Eo, then P = Esum*exp(-Nc)  (lands in Ei)
            nc.gpsimd.tensor_add(Ei[:], Ei[:], Ef[:])
            nc.gpsimd.tensor_add(Ei[:], Ei[:], Eo[:])
            ENc_bc = ENc[:].unsqueeze(2).broadcast_to([HC, B, tb])
            nc.vector.tensor_mul(r3(Ei[:]), r3(Ei[:]), ENc_bc)

            # delta scan: delta = (1-0.01P)*prev + (0.01P-0.03)
            a = chn.tile([128, nfd], F32, tag="a")
            nc.vector.tensor_scalar(a[:], Ei[:], -0.01, 1.0, OP.mult, OP.add)
            rr = chn.tile([128, nfd], F32, tag="r")
            nc.gpsimd.tensor_scalar(rr[:], Ei[:], 0.01, -0.03, OP.mult, OP.add)
            nc.vector.memset(r3(a[:])[:, :, 0], 0.0)
            d = chn.tile([128, nfd], F32, tag="d")
            nc.vector.tensor_tensor_scan(d[:], a[:], rr[:], 0.0, OP.mult, OP.add)
            nc.vector.tensor_add(Nc[:], Nc[:], r3(d[:])[:, :, tb - 1])

            # EN = exp(-(Nc_old + delta_{t-1})): shifted exp, slots = 1, * ENc
            ED = chn.tile([128, nfd], F32, tag="ED")
            nc.scalar.activation(ED[:, 1:nfd], d[:, 0 : nfd - 1], AF.Exp, scale=-1.0)
            nc.vector.memset(r3(ED[:])[:, :, 0], 1.0)
            nc.vector.tensor_mul(r3(ED[:]), r3(ED[:]), ENc_bc)

            # c scan: a_c = Ef*EN (in Ef), b_c = EiG*EN (in Tg)
            nc.vector.tensor_mul(Ef[:], Ef[:], ED[:])
            nc.vector.tensor_mul(Tg[:], Tg[:], ED[:])
            t64 = sm.tile([HC, B], F32, tag="t64")
            nc.vector.tensor_mul(t64[:], r3(Ef[:])[:, :, 0], Cc_v)
            nc.vector.tensor_add(
                r3(Tg[:])[:, :, 0], r3(Tg[:])[:, :, 0], t64[:]
            )
            nc.vector.memset(r3(Ef[:])[:, :, 0], 0.0)
            c = chn.tile([128, nfd], F32, tag="c")
            nc.vector.tensor_tensor_scan(c[:], Ef[:], Tg[:], 0.0, OP.mult, OP.add)

            # sigmoid(c) via tanh; L1 = (q-0.5)^2+0.75, q = DT/2*(Tl+1)
            Tc = chn.tile([128, nfd], F32, tag="Tc")
            nc.scalar.activation(Tc[:], c[:], AF.Tanh, scale=0.5)
            nc.vector.tensor_scalar(
                Tl[:], Tl[:], DT / 2, DT / 2, OP.mult, OP.add
            )
            L1 = chn.tile([128, nfd], F32, tag="L1")
            nc.scalar.activation(L1[:], Tl[:], AF.Square, bias=bm05[:])
            nc.vector.tensor_scalar(L1[:], L1[:], 0.75, None, OP.add)

            # b_h = (DT/2*Eo)*L1*EN*(Tc+1)   (lands in Eo)
            nc.gpsimd.tensor_scalar(Eo[:], Eo[:], DT / 2, None, OP.mult)
            nc.gpsimd.tensor_mul(Eo[:], Eo[:], L1[:])
            nc.vector.tensor_mul(Eo[:], Eo[:], ED[:])
            nc.gpsimd.tensor_scalar(Tc[:], Tc[:], 1.0, None, OP.add)
            nc.vector.tensor_mul(Eo[:], Eo[:], Tc[:])
            t64b = sm.tile([HC, B], F32, tag="t64b")
            nc.vector.tensor_mul(t64b[:], r3(L1[:])[:, :, 0], Hc_v)
            nc.vector.tensor_add(
                r3(Eo[:])[:, :, 0], r3(Eo[:])[:, :, 0], t64b[:]
            )
            nc.vector.memset(r3(L1[:])[:, :, 0], 0.0)
            h = chn.tile([128, nfd], F32, tag="h")
            nc.vector.tensor_tensor_scan(h[:], L1[:], Eo[:], 0.0, OP.mult, OP.add)

            # y partials: psum[m, 2j:2j+2] = h-slab_j.T @ projT
            ps = pp.tile([128, tb], F32)
            for j in range(nslab):
                nc.tensor.matmul(
                    ps[:, 2 * j : 2 * j + 2],
                    h[:, 128 * j : 128 * (j + 1)],
                    pj[:],
                    start=True,
                    stop=True,
                )
            ysb = sm.tile([128, tb], F32, tag="ysb")
            nc.scalar.copy(ysb[:], ps[:])
            nc.sync.dma_start(y_d[k], ysb[:])

            # c grows without bound for lanes with persistent f_t > 1 (the
            # reference saturates through sigmoid(inf)=1).  Clamp the carry so
            # the next block's 0*carry segment reset never sees inf; any clamp
            # >= ~30 leaves sigmoid(c) exactly 1.0f, and within-block growth
            # from 1e20 stays finite.
            Ccl = sm.tile([HC, B], F32, tag="ccl")
            nc.vector.tensor_scalar_min(Ccl[:], r3(c[:])[:, :, tb - 1], 1e20)
            Cc_v = Ccl[:]
            Hc_v = r3(h[:])[:, :, tb - 1]
            nc.scalar.activation(ENc[:], Nc[:], AF.Exp, scale=-1.0)

    nc.compile()
    return nc


def _get_program():
    key = (S, TB)
    if key not in _cached:
        _cached[key] = build_program(S, TB)
    return _cached[key]


def host_inputs(x_codes, Wi_w, Wi_b, Wf_w, Wf_b, Wo_w, Wo_b, Wg_w, Wg_b,
                Wl_w, Wl_b, proj_w, proj_b, n_init):
    """Fold input normalization into per-gate ACT scale/bias; shard over H."""
    f = lambda v: np.asarray(v, np.float32)
    cols = []
    for (w, b) in ((Wi_w, Wi_b), (Wf_w, Wf_b), (Wo_w, Wo_b)):
        cols += [f(w) / 100.0, f(b) - 0.65 * f(w)]
    for (w, b) in ((Wg_w, Wg_b), (Wl_w, Wl_b)):
        cols += [f(w) / 200.0, (f(b) - 0.65 * f(w)) / 2.0]
    wv_full = np.stack(cols, axis=1).astype(np.float32)  # [H, 10]
    x = np.ascontiguousarray(f(x_codes))
    pw = f(proj_w)
    n0 = f(n_init)
    maps = []
    for k in range(NCORES):
        hs = slice(k * HC, (k + 1) * HC)
        maps.append({
            "x": x,
            "wv": np.ascontiguousarray(wv_full[hs]),
            "projT": np.ascontiguousarray(pw[:, hs].T),
            "n0": np.ascontiguousarray(n0[hs].reshape(HC, 1)),
        })
    return maps


def assemble_output(results, proj_b, s=S, tb=TB):
    nb = s // tb
    nslab = (B * tb) // 128
    bper = 128 // tb  # batches per slab
    y = np.zeros((B, s, 2), np.float64)
    for k in range(NCORES):
        yc = np.asarray(results[k]["yout"], np.float64)
        ycr = yc.reshape(nb, bper, tb, nslab, 2)
        y += np.transpose(ycr, (3, 1, 0, 2, 4)).reshape(B, s, 2)
    y += np.asarray(proj_b, np.float64)[None, None, :]
    return y.astype(np.float32)


def kernel(**inputs):
    global _last_results
    nc = _get_program()
    maps = host_inputs(**inputs)
    res = run_bass_kernel_spmd(
        nc, maps, list(range(NCORES)),
        trace=bool(os.environ.get("KTRACE")),
        tmpdir=os.environ.get("KTRACE_DIR") or None,
    )
    _last_results = res
    return assemble_output(res.results, inputs["proj_b"])


# revision 13
# speedup vs baseline: 2.1161x; 2.1161x over previous
"""CfC head (mLSTM-style scan) Trainium2 kernel.

Math (per timestep t, per (b,h)):
    pre_g = xt*Wg_w + Wg_b            (xt = (x_codes-65)/100)
    i_t = exp(pre_i - n), f_t = exp(pre_f - n), o_t = exp(pre_o - n)
    g_t = sigmoid(pre_g); lam = sigmoid(pre_l)
    c   = f_t*c + i_t*g_t
    h   = (h + DT*o_t*sigmoid(c)) / (1 + DT*lam)
    n  += 0.01*(i_t + f_t + o_t - 3)
    y_t = h @ proj_w.T + proj_b

Device mapping: H=1024 sharded over 8 cores (128 h-values per core, one SBUF
partition each); free dim packs (batch-major, time-minor) blocks of TB steps.
The n-recurrence is handled per block by tracking the within-block drift
delta = n - n_blockstart, linearized as the affine scan
    delta_t = (1 - 0.01*P_t) * delta_{t-1} + (0.01*P_t - 0.03),
    P_t = (Ei+Ef+Eo)_t * exp(-n_blockstart),
which runs as one tensor_tensor_scan over the whole block (validated: rel err
1.4e-4 at TB=32 vs exact). c and h are exact affine scans given en = exp(-n):
    c_t = (Ef_t*en) * c_{t-1} + (Ei_t*G_t*en)
    h_t = L1_t * h_{t-1} + L1_t*DT*Eo_t*en*sigmoid(c_t),  L1 = 1/(1+DT*lam)
L1 uses the Neumann form 1 - q + q^2 = (q-0.5)^2 + 0.75 (q = DT*lam <= 0.01).
Sigmoids use tanh so every activation (exp/tanh/square) lives in the single
"exp_and_others" ACT table set (no table reloads).

y partials (over each core's 128 h) are accumulated on PE into PSUM and
summed across cores on the host.
"""

import os
from contextlib import ExitStack

import numpy as np

import concourse.bacc as bacc
import concourse.mybir as mybir
import concourse.tile as tile
from concourse.bass_utils import run_bass_kernel_spmd

AF = mybir.ActivationFunctionType
OP = mybir.AluOpType
F32 = mybir.dt.float32

B, S, H = 64, 2048, 1024
NCORES = 8
HC = H // NCORES  # 128 h-values per core = partition dim
DT = 0.01

TB = int(os.environ.get("KERNEL_TB", "32"))  # timesteps per block

_cached = {}
_last_results = None


def build_program(s=S, tb=TB):
    nb = s // tb
    nfd = B * tb           # free dim of block tiles, (b-major, t-minor)
    nslab = nfd // 128     # 128-wide matmul slabs per block

    nc = bacc.Bacc(
        "TRN2", target_bir_lowering=False, debug=False, num_devices=NCORES
    )
    x_d = nc.dram_tensor("x", [B, s], F32, kind="ExternalInput").ap()
    wv_d = nc.dram_tensor("wv", [HC, 10], F32, kind="ExternalInput").ap()
    pj_d = nc.dram_tensor("projT", [HC, 2], F32, kind="ExternalInput").ap()
    n0_d = nc.dram_tensor("n0", [HC, 1], F32, kind="ExternalInput").ap()
    y_d = nc.dram_tensor("yout", [nb, 128, tb], F32, kind="ExternalOutput").ap()

    def r3(ap):  # [128, nfd] -> [128, B, tb]
        return ap.rearrange("p (b t) -> p b t", t=tb)

    with tile.TileContext(nc) as tc, ExitStack() as ctx:
        wp = ctx.enter_context(tc.tile_pool(name="w", bufs=1))
        pha = ctx.enter_context(tc.tile_pool(name="pha", bufs=2))
        chn = ctx.enter_context(tc.tile_pool(name="chn", bufs=1))
        sm = ctx.enter_context(tc.tile_pool(name="sm", bufs=2))
        pp = ctx.enter_context(tc.tile_pool(name="pp", bufs=2, space="PSUM"))

        wv = wp.tile([HC, 10], F32)
        nc.sync.dma_start(wv[:], wv_d)
        pj = wp.tile([HC, 2], F32)
        nc.sync.dma_start(pj[:], pj_d)
        n0t = wp.tile([HC, 1], F32)
        nc.sync.dma_start(n0t[:], n0_d)

        # carries: n at block start (per h,b), exp(-n), c, h
        Nc = wp.tile([HC, B], F32)
        nc.vector.memset(Nc[:], 0.0)
        nc.vector.tensor_scalar(Nc[:], Nc[:], n0t[:, 0:1], None, OP.add)
        ENc = wp.tile([HC, B], F32)
        nc.scalar.activation(ENc[:], Nc[:], AF.Exp, scale=-1.0)
        cz = wp.tile([HC, B], F32)
        nc.vector.memset(cz[:], 0.0)
        hz = wp.tile([HC, B], F32)
        nc.vector.memset(hz[:], 0.0)
        Cc_v, Hc_v = cz[:], hz[:]
        bqm = wp.tile([HC, 1], F32)
        nc.vector.memset(bqm[:], DT / 2 - 0.5)

        for k in range(nb):
            t0 = k * tb
            X = pha.tile([128, nfd], F32, tag="X")
            nc.sync.dma_start(
                r3(X[:]), x_d[:, t0 : t0 + tb].partition_broadcast(128)
            )
            # gate pre-activations, fused through ACT scale/bias
            Ei = pha.tile([128, nfd], F32, tag="Ei")
            nc.scalar.activation(
                Ei[:], X[:], AF.Exp, bias=wv[:, 1:2], scale=wv[:, 0:1]
            )
            Ef = pha.tile([128, nfd], F32, tag="Ef")
            nc.scalar.activation(
                Ef[:], X[:], AF.Exp, bias=wv[:, 3:4], scale=wv[:, 2:3]
            )
            Eo = pha.tile([128, nfd], F32, tag="Eo")
            nc.scalar.activation(
                Eo[:], X[:], AF.Exp, bias=wv[:, 5:6], scale=wv[:, 4:5]
            )
            Tg = pha.tile([128, nfd], F32, tag="Tg")
            nc.scalar.activation(
                Tg[:], X[:], AF.Tanh, bias=wv[:, 7:8], scale=wv[:, 6:7]
            )
            Tl = pha.tile([128, nfd], F32, tag="Tl")
            nc.scalar.activation(
                Tl[:], X[:], AF.Tanh, bias=wv[:, 9:10], scale=wv[:, 8:9]
            )

            # G = 0.5*Tg+0.5 ; EiG = Ei*G  (both land in Tg)
            nc.vector.tensor_scalar(Tg[:], Tg[:], 0.5, 0.5, OP.mult, OP.add)
            nc.vector.tensor_mul(Tg[:], Ei[:], Tg[:])
            # Esum = Ei+Ef+Eo, then P = Esum*exp(-Nc)  (lands in Ei)
            nc.vector.tensor_add(Ei[:], Ei[:], Ef[:])
            nc.vector.tensor_add(Ei[:], Ei[:], Eo[:])
            ENc_bc = ENc[:].unsqueeze(2).broadcast_to([HC, B, tb])
            nc.vector.tensor_mul(r3(Ei[:]), r3(Ei[:]), ENc_bc)

            # delta scan: delta = (1-0.01P)*prev + (0.01P-0.03)
            a = chn.tile([128, nfd], F32, tag="a")
            nc.vector.tensor_scalar(a[:], Ei[:], -0.01, 1.0, OP.mult, OP.add)
            rr = chn.tile([128, nfd], F32, tag="r")
            nc.vector.tensor_scalar(rr[:], Ei[:], 0.01, -0.03, OP.mult, OP.add)
            nc.vector.memset(r3(a[:])[:, :, 0], 0.0)
            d = chn.tile([128, nfd], F32, tag="d")
            nc.vector.tensor_tensor_scan(d[:], a[:], rr[:], 0.0, OP.mult, OP.add)
            nc.vector.tensor_add(Nc[:], Nc[:], r3(d[:])[:, :, tb - 1])

            # EN = exp(-(Nc_old + delta_{t-1})): shifted exp, slots = 1, * ENc
            ED = chn.tile([128, nfd], F32, tag="ED")
            nc.scalar.activation(ED[:, 1:nfd], d[:, 0 : nfd - 1], AF.Exp, scale=-1.0)
            nc.vector.memset(r3(ED[:])[:, :, 0], 1.0)
            nc.vector.tensor_mul(r3(ED[:]), r3(ED[:]), ENc_bc)

            # c scan: a_c = Ef*EN (in Ef), b_c = EiG*EN (in Tg)
            nc.vector.tensor_mul(Ef[:], Ef[:], ED[:])
            nc.vector.tensor_mul(Tg[:], Tg[:], ED[:])
            t64 = sm.tile([HC, B], F32, tag="t64")
            nc.vector.tensor_mul(t64[:], r3(Ef[:])[:, :, 0], Cc_v)
            nc.vector.tensor_add(
                r3(Tg[:])[:, :, 0], r3(Tg[:])[:, :, 0], t64[:]
            )
            nc.vector.memset(r3(Ef[:])[:, :, 0], 0.0)
            c = chn.tile([128, nfd], F32, tag="c")
            nc.vector.tensor_tensor_scan(c[:], Ef[:], Tg[:], 0.0, OP.mult, OP.add)

            # sigmoid(c) via tanh; L1 = 1-q+q^2 = (q-0.5)^2+0.75 with
            # q = DT*lam = DT/2*(Tl+1): fold q into the Square ACT directly:
            # Sq = (DT/2*Tl + (DT/2-0.5))^2
            Tc = chn.tile([128, nfd], F32, tag="Tc")
            nc.scalar.activation(Tc[:], c[:], AF.Tanh, scale=0.5)
            Sq = chn.tile([128, nfd], F32, tag="Sq")
            nc.scalar.activation(Sq[:], Tl[:], AF.Square, bias=bqm[:], scale=DT / 2)
            L1 = chn.tile([128, nfd], F32, tag="L1")
            nc.vector.tensor_scalar(L1[:], Sq[:], 0.75, None, OP.add)
            # L1D = DT/2 * L1, straight from Sq
            nc.vector.tensor_scalar(Sq[:], Sq[:], 0.75, DT / 2, OP.add, OP.mult)

            # b_h = Eo*L1D*EN*(Tc+1)   (lands in Eo)
            nc.vector.tensor_mul(Eo[:], Eo[:], Sq[:])
            nc.vector.tensor_mul(Eo[:], Eo[:], ED[:])
            nc.vector.tensor_scalar(Tc[:], Tc[:], 1.0, None, OP.add)
            nc.vector.tensor_mul(Eo[:], Eo[:], Tc[:])
            t64b = sm.tile([HC, B], F32, tag="t64b")
            nc.vector.tensor_mul(t64b[:], r3(L1[:])[:, :, 0], Hc_v)
            nc.vector.tensor_add(
                r3(Eo[:])[:, :, 0], r3(Eo[:])[:, :, 0], t64b[:]
            )
            nc.vector.memset(r3(L1[:])[:, :, 0], 0.0)
            h = chn.tile([128, nfd], F32, tag="h")
            nc.vector.tensor_tensor_scan(h[:], L1[:], Eo[:], 0.0, OP.mult, OP.add)

            # y partials: psum[m, 2j:2j+2] = h-slab_j.T @ projT
            ps = pp.tile([128, tb], F32)
            for j in range(nslab):
                nc.tensor.matmul(
                    ps[:, 2 * j : 2 * j + 2],
                    h[:, 128 * j : 128 * (j + 1)],
                    pj[:],
                    start=True,
                    stop=True,
                )
            ysb = sm.tile([128, tb], F32, tag="ysb")
            nc.scalar.copy(ysb[:], ps[:])
            nc.sync.dma_start(y_d[k], ysb[:])

            # c grows without bound for lanes with persistent f_t > 1 (the
            # reference saturates through sigmoid(inf)=1).  Clamp the carry so
            # the next block's 0*carry segment reset never sees inf; any clamp
            # >= ~30 leaves sigmoid(c) exactly 1.0f, and within-block growth
            # from 1e20 stays finite.
            Ccl = sm.tile([HC, B], F32, tag="ccl")
            nc.vector.tensor_scalar_min(Ccl[:], r3(c[:])[:, :, tb - 1], 1e20)
            Cc_v = Ccl[:]
            Hc_v = r3(h[:])[:, :, tb - 1]
            nc.scalar.activation(ENc[:], Nc[:], AF.Exp, scale=-1.0)

    nc.compile()
    return nc


def _get_program():
    key = (S, TB)
    if key not in _cached:
        _cached[key] = build_program(S, TB)
    return _cached[key]


def host_inputs(x_codes, Wi_w, Wi_b, Wf_w, Wf_b, Wo_w, Wo_b, Wg_w, Wg_b,
                Wl_w, Wl_b, proj_w, proj_b, n_init):
    """Fold input normalization into per-gate ACT scale/bias; shard over H."""
    f = lambda v: np.asarray(v, np.float32)
    cols = []
    for (w, b) in ((Wi_w, Wi_b), (Wf_w, Wf_b), (Wo_w, Wo_b)):
        cols += [f(w) / 100.0, f(b) - 0.65 * f(w)]
    for (w, b) in ((Wg_w, Wg_b), (Wl_w, Wl_b)):
        cols += [f(w) / 200.0, (f(b) - 0.65 * f(w)) / 2.0]
    wv_full = np.stack(cols, axis=1).astype(np.float32)  # [H, 10]
    x = np.ascontiguousarray(f(x_codes))
    pw = f(proj_w)
    n0 = f(n_init)
    maps = []
    for k in range(NCORES):
        hs = slice(k * HC, (k + 1) * HC)
        maps.append({
            "x": x,
            "wv": np.ascontiguousarray(wv_full[hs]),
            "projT": np.ascontiguousarray(pw[:, hs].T),
            "n0": np.ascontiguousarray(n0[hs].reshape(HC, 1)),
        })
    return maps


def assemble_output(results, proj_b, s=S, tb=TB):
    nb = s // tb
    nslab = (B * tb) // 128
    bper = 128 // tb  # batches per slab
    y = np.zeros((B, s, 2), np.float64)
    for k in range(NCORES):
        yc = np.asarray(results[k]["yout"], np.float64)
        ycr = yc.reshape(nb, bper, tb, nslab, 2)
        y += np.transpose(ycr, (3, 1, 0, 2, 4)).reshape(B, s, 2)
    y += np.asarray(proj_b, np.float64)[None, None, :]
    return y.astype(np.float32)


def kernel(**inputs):
    global _last_results
    nc = _get_program()
    maps = host_inputs(**inputs)
    res = run_bass_kernel_spmd(
        nc, maps, list(range(NCORES)),
        trace=bool(os.environ.get("KTRACE")),
        tmpdir=os.environ.get("KTRACE_DIR") or None,
    )
    _last_results = res
    return assemble_output(res.results, inputs["proj_b"])


# revision 14
# speedup vs baseline: 2.1325x; 1.0078x over previous
"""CfC head (mLSTM-style scan) Trainium2 kernel.

Math (per timestep t, per (b,h)):
    pre_g = xt*Wg_w + Wg_b            (xt = (x_codes-65)/100)
    i_t = exp(pre_i - n), f_t = exp(pre_f - n), o_t = exp(pre_o - n)
    g_t = sigmoid(pre_g); lam = sigmoid(pre_l)
    c   = f_t*c + i_t*g_t
    h   = (h + DT*o_t*sigmoid(c)) / (1 + DT*lam)
    n  += 0.01*(i_t + f_t + o_t - 3)
    y_t = h @ proj_w.T + proj_b

Device mapping: H=1024 sharded over 8 cores (128 h-values per core, one SBUF
partition each); free dim packs (batch-major, time-minor) blocks of TB steps.
The n-recurrence is handled per block by tracking the within-block drift
delta = n - n_blockstart, linearized as the affine scan
    delta_t = (1 - 0.01*P_t) * delta_{t-1} + (0.01*P_t - 0.03),
    P_t = (Ei+Ef+Eo)_t * exp(-n_blockstart),
which runs as one tensor_tensor_scan over the whole block (validated: rel err
1.4e-4 at TB=32 vs exact). c and h are exact affine scans given en = exp(-n):
    c_t = (Ef_t*en) * c_{t-1} + (Ei_t*G_t*en)
    h_t = L1_t * h_{t-1} + L1_t*DT*Eo_t*en*sigmoid(c_t),  L1 = 1/(1+DT*lam)
L1 uses the Neumann form 1 - q + q^2 = (q-0.5)^2 + 0.75 (q = DT*lam <= 0.01).
Sigmoids use tanh so every activation (exp/tanh/square) lives in the single
"exp_and_others" ACT table set (no table reloads).

y partials (over each core's 128 h) are accumulated on PE into PSUM and
summed across cores on the host.
"""

import os
from contextlib import ExitStack

import numpy as np

import concourse.bacc as bacc
import concourse.mybir as mybir
import concourse.tile as tile
from concourse.bass_utils import run_bass_kernel_spmd

AF = mybir.ActivationFunctionType
OP = mybir.AluOpType
F32 = mybir.dt.float32

B, S, H = 64, 2048, 1024
NCORES = 8
HC = H // NCORES  # 128 h-values per core = partition dim
DT = 0.01

TB = int(os.environ.get("KERNEL_TB", "32"))  # timesteps per block

_cached = {}
_last_results = None


def build_program(s=S, tb=TB):
    nb = s // tb
    nfd = B * tb           # free dim of block tiles, (b-major, t-minor)
    nslab = nfd // 128     # 128-wide matmul slabs per block

    nc = bacc.Bacc(
        "TRN2", target_bir_lowering=False, debug=False, num_devices=NCORES
    )
    x_d = nc.dram_tensor("x", [B, s], F32, kind="ExternalInput").ap()
    wv_d = nc.dram_tensor("wv", [HC, 10], F32, kind="ExternalInput").ap()
    pj_d = nc.dram_tensor("projT", [HC, 2], F32, kind="ExternalInput").ap()
    n0_d = nc.dram_tensor("n0", [HC, 1], F32, kind="ExternalInput").ap()
    y_d = nc.dram_tensor("yout", [nb, 128, tb], F32, kind="ExternalOutput").ap()

    def r3(ap):  # [128, nfd] -> [128, B, tb]
        return ap.rearrange("p (b t) -> p b t", t=tb)

    with tile.TileContext(nc) as tc, ExitStack() as ctx:
        wp = ctx.enter_context(tc.tile_pool(name="w", bufs=1))
        pha = ctx.enter_context(tc.tile_pool(name="pha", bufs=2))
        chn = ctx.enter_context(tc.tile_pool(name="chn", bufs=1))
        sm = ctx.enter_context(tc.tile_pool(name="sm", bufs=2))
        pp = ctx.enter_context(tc.tile_pool(name="pp", bufs=2, space="PSUM"))

        wv = wp.tile([HC, 10], F32)
        nc.sync.dma_start(wv[:], wv_d)
        pj = wp.tile([HC, 2], F32)
        nc.sync.dma_start(pj[:], pj_d)
        n0t = wp.tile([HC, 1], F32)
        nc.sync.dma_start(n0t[:], n0_d)

        # carries: n at block start (per h,b), exp(-n), c, h
        Nc = wp.tile([HC, B], F32)
        nc.vector.memset(Nc[:], 0.0)
        nc.vector.tensor_scalar(Nc[:], Nc[:], n0t[:, 0:1], None, OP.add)
        ENc = wp.tile([HC, B], F32)
        nc.scalar.activation(ENc[:], Nc[:], AF.Exp, scale=-1.0)
        cz = wp.tile([HC, B], F32)
        nc.vector.memset(cz[:], 0.0)
        hz = wp.tile([HC, B], F32)
        nc.vector.memset(hz[:], 0.0)
        Cc_v, Hc_v = cz[:], hz[:]
        bqm = wp.tile([HC, 1], F32)
        nc.vector.memset(bqm[:], DT / 2 - 0.5)

        for k in range(nb):
            t0 = k * tb
            X = pha.tile([128, nfd], F32, tag="X")
            nc.sync.dma_start(
                r3(X[:]), x_d[:, t0 : t0 + tb].partition_broadcast(128)
            )
            # gate pre-activations, fused through ACT scale/bias
            Ei = pha.tile([128, nfd], F32, tag="Ei")
            nc.scalar.activation(
                Ei[:], X[:], AF.Exp, bias=wv[:, 1:2], scale=wv[:, 0:1]
            )
            Ef = pha.tile([128, nfd], F32, tag="Ef")
            nc.scalar.activation(
                Ef[:], X[:], AF.Exp, bias=wv[:, 3:4], scale=wv[:, 2:3]
            )
            Eo = pha.tile([128, nfd], F32, tag="Eo")
            nc.scalar.activation(
                Eo[:], X[:], AF.Exp, bias=wv[:, 5:6], scale=wv[:, 4:5]
            )
            Tg = pha.tile([128, nfd], F32, tag="Tg")
            nc.scalar.activation(
                Tg[:], X[:], AF.Tanh, bias=wv[:, 7:8], scale=wv[:, 6:7]
            )
            Tl = pha.tile([128, nfd], F32, tag="Tl")
            nc.scalar.activation(
                Tl[:], X[:], AF.Tanh, bias=wv[:, 9:10], scale=wv[:, 8:9]
            )

            # G = 0.5*Tg+0.5 ; EiG = Ei*G  (both land in Tg)
            nc.vector.tensor_scalar(Tg[:], Tg[:], 0.5, 0.5, OP.mult, OP.add)
            nc.vector.tensor_mul(Tg[:], Ei[:], Tg[:])
            # Esum = Ei+Ef+Eo, then P = Esum*exp(-Nc)  (lands in Ei)
            nc.vector.tensor_add(Ei[:], Ei[:], Ef[:])
            nc.vector.tensor_add(Ei[:], Ei[:], Eo[:])
            ENc_bc = ENc[:].unsqueeze(2).broadcast_to([HC, B, tb])
            nc.vector.tensor_mul(r3(Ei[:]), r3(Ei[:]), ENc_bc)

            # delta scan: delta = (1-0.01P)*prev + (0.01P-0.03)
            a = chn.tile([128, nfd], F32, tag="a")
            nc.vector.tensor_scalar(a[:], Ei[:], -0.01, 1.0, OP.mult, OP.add)
            rr = chn.tile([128, nfd], F32, tag="r")
            nc.vector.tensor_scalar(rr[:], Ei[:], 0.01, -0.03, OP.mult, OP.add)
            nc.vector.memset(r3(a[:])[:, :, 0], 0.0)
            d = chn.tile([128, nfd], F32, tag="d")
            nc.vector.tensor_tensor_scan(d[:], a[:], rr[:], 0.0, OP.mult, OP.add)
            nc.vector.tensor_add(Nc[:], Nc[:], r3(d[:])[:, :, tb - 1])

            # EN = exp(-(Nc_old + delta_{t-1})): shifted exp, slots = 1, * ENc
            ED = chn.tile([128, nfd], F32, tag="ED")
            nc.scalar.activation(ED[:, 1:nfd], d[:, 0 : nfd - 1], AF.Exp, scale=-1.0)
            nc.vector.memset(r3(ED[:])[:, :, 0], 1.0)
            nc.vector.tensor_mul(r3(ED[:]), r3(ED[:]), ENc_bc)

            # c scan: a_c = Ef*EN (in Ef), b_c = EiG*EN (in Tg)
            nc.vector.tensor_mul(Ef[:], Ef[:], ED[:])
            nc.vector.tensor_mul(Tg[:], Tg[:], ED[:])
            t64 = sm.tile([HC, B], F32, tag="t64")
            nc.vector.tensor_mul(t64[:], r3(Ef[:])[:, :, 0], Cc_v)
            nc.vector.tensor_add(
                r3(Tg[:])[:, :, 0], r3(Tg[:])[:, :, 0], t64[:]
            )
            nc.vector.memset(r3(Ef[:])[:, :, 0], 0.0)
            c = chn.tile([128, nfd], F32, tag="c")
            nc.vector.tensor_tensor_scan(c[:], Ef[:], Tg[:], 0.0, OP.mult, OP.add)

            # sigmoid(c) via tanh; L1 = 1-q+q^2 = (q-0.5)^2+0.75 with
            # q = DT*lam = DT/2*(Tl+1): fold q into the Square ACT directly:
            # Sq = (DT/2*Tl + (DT/2-0.5))^2
            Tc = chn.tile([128, nfd], F32, tag="Tc")
            nc.scalar.activation(Tc[:], c[:], AF.Tanh, scale=0.5)
            Sq = chn.tile([128, nfd], F32, tag="Sq")
            nc.scalar.activation(Sq[:], Tl[:], AF.Square, bias=bqm[:], scale=DT / 2)
            L1 = chn.tile([128, nfd], F32, tag="L1")
            nc.vector.tensor_scalar(L1[:], Sq[:], 0.75, None, OP.add)
            # L1D = DT/2 * L1, straight from Sq (own tile: keep both reads of
            # Sq independent so neither serializes the other)
            L1D = chn.tile([128, nfd], F32, tag="L1D")
            nc.vector.tensor_scalar(L1D[:], Sq[:], 0.75, DT / 2, OP.add, OP.mult)

            # b_h = Eo*L1D*EN*(Tc+1)   (lands in Eo)
            nc.vector.tensor_mul(Eo[:], Eo[:], L1D[:])
            nc.vector.tensor_mul(Eo[:], Eo[:], ED[:])
            nc.vector.tensor_scalar(Tc[:], Tc[:], 1.0, None, OP.add)
            nc.vector.tensor_mul(Eo[:], Eo[:], Tc[:])
            t64b = sm.tile([HC, B], F32, tag="t64b")
            nc.vector.tensor_mul(t64b[:], r3(L1[:])[:, :, 0], Hc_v)
            nc.vector.tensor_add(
                r3(Eo[:])[:, :, 0], r3(Eo[:])[:, :, 0], t64b[:]
            )
            nc.vector.memset(r3(L1[:])[:, :, 0], 0.0)
            h = chn.tile([128, nfd], F32, tag="h")
            nc.vector.tensor_tensor_scan(h[:], L1[:], Eo[:], 0.0, OP.mult, OP.add)

            # y partials: psum[m, 2j:2j+2] = h-slab_j.T @ projT
            ps = pp.tile([128, tb], F32)
            for j in range(nslab):
                nc.tensor.matmul(
                    ps[:, 2 * j : 2 * j + 2],
                    h[:, 128 * j : 128 * (j + 1)],
                    pj[:],
                    start=True,
                    stop=True,
                )
            ysb = sm.tile([128, tb], F32, tag="ysb")
            nc.scalar.copy(ysb[:], ps[:])
            nc.sync.dma_start(y_d[k], ysb[:])

            # c grows without bound for lanes with persistent f_t > 1 (the
            # reference saturates through sigmoid(inf)=1).  Clamp the carry so
            # the next block's 0*carry segment reset never sees inf; any clamp
            # >= ~30 leaves sigmoid(c) exactly 1.0f, and within-block growth
            # from 1e20 stays finite.
            Ccl = sm.tile([HC, B], F32, tag="ccl")
            nc.vector.tensor_scalar_min(Ccl[:], r3(c[:])[:, :, tb - 1], 1e20)
            Cc_v = Ccl[:]
            Hc_v = r3(h[:])[:, :, tb - 1]
            nc.scalar.activation(ENc[:], Nc[:], AF.Exp, scale=-1.0)

    nc.compile()
    return nc


def _get_program():
    key = (S, TB)
    if key not in _cached:
        _cached[key] = build_program(S, TB)
    return _cached[key]


def host_inputs(x_codes, Wi_w, Wi_b, Wf_w, Wf_b, Wo_w, Wo_b, Wg_w, Wg_b,
                Wl_w, Wl_b, proj_w, proj_b, n_init):
    """Fold input normalization into per-gate ACT scale/bias; shard over H."""
    f = lambda v: np.asarray(v, np.float32)
    cols = []
    for (w, b) in ((Wi_w, Wi_b), (Wf_w, Wf_b), (Wo_w, Wo_b)):
        cols += [f(w) / 100.0, f(b) - 0.65 * f(w)]
    for (w, b) in ((Wg_w, Wg_b), (Wl_w, Wl_b)):
        cols += [f(w) / 200.0, (f(b) - 0.65 * f(w)) / 2.0]
    wv_full = np.stack(cols, axis=1).astype(np.float32)  # [H, 10]
    x = np.ascontiguousarray(f(x_codes))
    pw = f(proj_w)
    n0 = f(n_init)
    maps = []
    for k in range(NCORES):
        hs = slice(k * HC, (k + 1) * HC)
        maps.append({
            "x": x,
            "wv": np.ascontiguousarray(wv_full[hs]),
            "projT": np.ascontiguousarray(pw[:, hs].T),
            "n0": np.ascontiguousarray(n0[hs].reshape(HC, 1)),
        })
    return maps


def assemble_output(results, proj_b, s=S, tb=TB):
    nb = s // tb
    nslab = (B * tb) // 128
    bper = 128 // tb  # batches per slab
    y = np.zeros((B, s, 2), np.float64)
    for k in range(NCORES):
        yc = np.asarray(results[k]["yout"], np.float64)
        ycr = yc.reshape(nb, bper, tb, nslab, 2)
        y += np.transpose(ycr, (3, 1, 0, 2, 4)).reshape(B, s, 2)
    y += np.asarray(proj_b, np.float64)[None, None, :]
    return y.astype(np.float32)


def kernel(**inputs):
    global _last_results
    nc = _get_program()
    maps = host_inputs(**inputs)
    res = run_bass_kernel_spmd(
        nc, maps, list(range(NCORES)),
        trace=bool(os.environ.get("KTRACE")),
        tmpdir=os.environ.get("KTRACE_DIR") or None,
    )
    _last_results = res
    return assemble_output(res.results, inputs["proj_b"])
